# revision 36
# baseline (speedup 1.0000x reference)
"""Trainium2 Bass kernel for nn_NeuralNet_19516331393457 (dense_mlp).

Pipeline: x = embed[data] (48-entry table); h1 = relu(x@W1+b1);
h2 = tanh(h1@W2+b2); out = h2@W3+b3; return out[argmax(F(out0, out1))].

Strategy (data-parallel over N=500000 on 8 cores), fp16 on device:
  - Host: tiny-table gather embed[data] in fp16 fused with a tile-blocked
    transpose; 16 up-front segment DMAs with 8 KiB DRAM lines stream the
    whole shard into SBUF (~124 KiB/partition) at full HBM bandwidth.
  - Device (per core, 63488 padded samples = 62 pairs of 2x512 chunks):
    MM1+relu and pair-packed MM2+tanh for all pairs; the tiny out=h2@W3
    is HYBRID: pairs 0..45 ship h2 (fp16 slabs, host does h2@W3 -- DRAM
    write bandwidth ~254 GB/s absorbs 6 MB while the PE works), pairs
    46..61 run MM3 on-device (their h2 is born too late to drain, so
    they go out as 0.4 MB of packed fp32->fp16 outs instead of 2 MB).
      * software-pipelined, 1-2 iteration skew; in-order PE queue sees
        only aged dependencies; stationaries grouped (w3,w2A,w2B,w1)
      * MM2 pair-packed: chunk A -> PSUM rows 0:64, chunk B -> rows
        64:128 (dup W2 at PE col groups 0/64); one tanh (+b2 stacked)
        evicts both chunks
      * device MM3 pair-packed twice: stacked h2 x block-diag W3 slab ->
        [4,512] at PSUM partition base {0,32,64}; one bank collects 3
        pairs; 6 copies total
      * relu/copies statically balanced between DVE and ACT
  - Host: decode both out paths, out = h2 @ W3 in fp32 for the slab part,
    F in float64, global argmax, return out[argmax] + b3.
"""

import numpy as np

import concourse.mybir as mybir
import concourse.tile as tile
from concourse import bacc
from concourse.bass_utils import run_bass_kernel_spmd

N = 500000
D = 128
H1 = 128
H2 = 64
NCLS = 2
NCORES = 8
CHUNK = 512
NPC_RAW = N // NCORES              # 62500 samples per core
PAIRS = 62                         # pairs of 2 chunks per core
CHUNKS = 2 * PAIRS                 # 124 chunks
NPC = CHUNKS * CHUNK               # 63488 padded samples per core
SEGPAIRS = 4                       # pairs per input DMA segment
SEGS = -(-PAIRS // SEGPAIRS)       # 16 segments (8 KiB DRAM lines each)
HOSTPAIRS = 46                     # pairs whose h2 ships to the host
DEVPAIRS = PAIRS - HOSTPAIRS       # 16 pairs with on-device MM3
SLABPAIRS = 8                      # pairs per output slab
SLABS = -(-HOSTPAIRS // SLABPAIRS)  # 6 slabs (last holds 6 pairs)
GROUPS = -(-DEVPAIRS // 3)         # 6 po groups (3 pairs per PSUM bank)

_F16 = mybir.dt.float16
_F32 = mybir.dt.float32

# measured per-op eviction costs (ns) for static DVE/ACT load balancing
_DVE_RELU, _ACT_RELU = 669.0, 638.0
_DVE_COPY, _ACT_COPY = 700.0, 650.0
_ACT_TANH = 638.0


def _build_bass():
    nc = bacc.Bacc(
        "TRN2",
        target_bir_lowering=False,
        debug=False,
        enable_asserts=False,
        num_devices=NCORES,
    )
    x_t = nc.dram_tensor("x_t", [SEGS, D, SEGPAIRS * 2 * CHUNK], _F16,
                         kind="ExternalInput")
    w1 = nc.dram_tensor("w1", [D, H1], _F16, kind="ExternalInput")
    w2d = nc.dram_tensor("w2d", [H1, 2 * H2], _F16, kind="ExternalInput")
    w3b = nc.dram_tensor("w3b", [2 * H2, 128], _F16, kind="ExternalInput")
    b1 = nc.dram_tensor("b1", [H1, 1], _F32, kind="ExternalInput")
    b2s = nc.dram_tensor("b2s", [2 * H2, 1], _F32, kind="ExternalInput")
    out_d = nc.dram_tensor("out_d", [SLABS, 128, SLABPAIRS * CHUNK], _F16,
                           kind="ExternalOutput")
    out0_d = nc.dram_tensor("out0_d", [GROUPS, 68, CHUNK], _F16,
                            kind="ExternalOutput")

    load = {"dve": 0.0, "act": 0.0}

    with tile.TileContext(nc) as tc:
        with (
            tc.tile_pool(name="w", bufs=1) as wpool,
            tc.tile_pool(name="x", bufs=SEGS) as xpool,
            tc.tile_pool(name="h1", bufs=8) as h1pool,
            tc.tile_pool(name="h2", bufs=4) as h2pool,
            tc.tile_pool(name="st", bufs=SLABS) as stpool,
            tc.tile_pool(name="ob", bufs=GROUPS) as obpool,
            tc.tile_pool(name="p1", bufs=4, space="PSUM") as p1pool,
            tc.tile_pool(name="p2", bufs=2, space="PSUM") as p2pool,
            tc.tile_pool(name="po", bufs=2, space="PSUM") as popool,
        ):
            w1sb = wpool.tile([D, H1], _F16)
            nc.sync.dma_start(w1sb[:], w1[:, :])
            w2sb = wpool.tile([H1, 2 * H2], _F16)
            nc.sync.dma_start(w2sb[:], w2d[:, :])
            w3sb = wpool.tile([2 * H2, 128], _F16)
            nc.sync.dma_start(w3sb[:], w3b[:, :])
            b1sb = wpool.tile([H1, 1], _F32)
            nc.sync.dma_start(b1sb[:], b1[:, :])
            b2sb = wpool.tile([2 * H2, 1], _F32)
            nc.sync.dma_start(b2sb[:], b2s[:, :])

            # prefetch the ACT table set (relu/tanh share one) under the
            # first input DMAs
            warm = wpool.tile([H1, 1], _F32)
            nc.scalar.activation(warm[:], b1sb[:],
                                 mybir.ActivationFunctionType.Relu)

            xsegs = []
            for s in range(SEGS):
                xt = xpool.tile([D, SEGPAIRS * 2 * CHUNK], _F16,
                                name=f"xseg{s}", tag="xt")
                if s == 0:
                    # per-pair slices so the first MM1 starts as soon as
                    # pair 0 lands, not after the whole 1 MiB segment
                    for q in range(SEGPAIRS):
                        sl = slice(q * 2 * CHUNK, (q + 1) * 2 * CHUNK)
                        nc.sync.dma_start(xt[:, sl], x_t[s, :, sl])
                else:
                    nc.sync.dma_start(xt[:], x_t[s, :, :])
                xsegs.append(xt)

            slabs = [
                stpool.tile([128, SLABPAIRS * CHUNK], _F16,
                            name=f"slab{o}", tag="st")
                for o in range(SLABS)
            ]

            p1s = {}     # chunk -> [128, 512] f32 psum
            h1s = {}     # chunk -> [128, 512] f16
            p2s = {}     # pair -> [128, 512] f32 psum (A rows 0:64, B 64:128)
            h2s = {}     # device pair -> [128, 512] f16
            pos = {}     # po group -> [128, 512] f32 psum

            def emit_mm1(p):
                seg, off = divmod(p, SEGPAIRS)
                xt = xsegs[seg]
                for half in range(2):
                    c = 2 * p + half
                    p1 = p1pool.tile([H1, CHUNK], _F32, name=f"p1_{c}",
                                     tag="p1")
                    base = off * 2 * CHUNK + half * CHUNK
                    nc.tensor.matmul(
                        p1[:], w1sb[:], xt[:, base:base + CHUNK],
                        start=True, stop=True)
                    p1s[c] = p1

            def emit_relu(p):
                for half in range(2):
                    c = 2 * p + half
                    h1t = h1pool.tile([H1, CHUNK], _F16, name=f"h1_{c}",
                                      tag="h1")
                    if load["act"] + _ACT_RELU <= load["dve"] + _DVE_RELU:
                        load["act"] += _ACT_RELU
                        nc.scalar.activation(
                            h1t[:], p1s[c][:],
                            mybir.ActivationFunctionType.Relu, bias=b1sb[:])
                    else:
                        load["dve"] += _DVE_RELU
                        nc.vector.tensor_scalar(
                            h1t[:], p1s[c][:], b1sb[:], 0.0,
                            mybir.AluOpType.add, mybir.AluOpType.max)
                    h1s[c] = h1t
                    del p1s[c]

            def emit_mm2(p, col):
                # col 0: chunk A -> rows 0:64; col 1: chunk B -> rows 64:128
                if col == 0:
                    p2 = p2pool.tile([128, CHUNK], _F32, name=f"p2_{p}",
                                     tag="p2")
                    p2s[p] = p2
                nc.tensor.matmul(
                    p2s[p][col * H2:(col + 1) * H2, :],
                    w2sb[:, col * H2:(col + 1) * H2],
                    h1s[2 * p + col][:], start=True, stop=True)

            def emit_tanh(p):
                if p < HOSTPAIRS:
                    o, j = divmod(p, SLABPAIRS)
                    dst = slabs[o][:, j * CHUNK:(j + 1) * CHUNK]
                else:
                    h2t = h2pool.tile([128, CHUNK], _F16, name=f"h2_{p}",
                                      tag="h2")
                    h2s[p] = h2t
                    dst = h2t[:]
                load["act"] += _ACT_TANH
                nc.scalar.activation(
                    dst, p2s[p][:], mybir.ActivationFunctionType.Tanh,
                    bias=b2sb[:])
                del p2s[p]
                if p < HOSTPAIRS and (p == HOSTPAIRS - 1
                                      or j == SLABPAIRS - 1):
                    w = (j + 1) * CHUNK
                    nc.sync.dma_start(out_d[o, :, 0:w], slabs[o][:, 0:w])

            def emit_mm3(p):
                g, m = divmod(p - HOSTPAIRS, 3)
                last = (m == 2) or (p == PAIRS - 1)
                if m == 0:
                    pos[g] = popool.tile([128, CHUNK], _F32,
                                         name=f"po{g}", tag="po")
                nc.tensor.matmul(
                    pos[g][32 * m:32 * m + 4, :],
                    w3sb[:, 32 * m:32 * m + 4], h2s[p][:],
                    start=True, stop=True)
                del h2s[p]
                if last:
                    ob = obpool.tile([68, CHUNK], _F16, name=f"ob{g}",
                                     tag="ob")
                    if load["act"] + _ACT_COPY <= load["dve"] + _DVE_COPY:
                        load["act"] += _ACT_COPY
                        nc.scalar.copy(ob[:], pos[g][0:68, :])
                    else:
                        load["dve"] += _DVE_COPY
                        nc.vector.tensor_copy(ob[:], pos[g][0:68, :])
                    nc.sync.dma_start(out0_d[g, :, :], ob[:])
                    del pos[g]

            # per-iteration skew: MM3(b-2) [device pairs], MM2(b-1),
            # MM1(b) on the PE; tanh(b-1) then relu(b) on the eviction
            # engines.  All PE deps are >=1 iteration old.
            NB = PAIRS // 2
            for b in range(NB + 3):
                bm3 = b - 2
                if 0 <= bm3 < NB and 2 * bm3 + 1 >= HOSTPAIRS:
                    for p in (2 * bm3, 2 * bm3 + 1):
                        if p >= HOSTPAIRS:
                            emit_mm3(p)
                if 0 <= b - 1 < NB:
                    for col in range(2):
                        emit_mm2(2 * (b - 1), col)
                        emit_mm2(2 * (b - 1) + 1, col)
                if b < NB:
                    emit_mm1(2 * b)
                    emit_mm1(2 * b + 1)
                if 0 <= b - 1 < NB:
                    emit_tanh(2 * (b - 1))
                    emit_tanh(2 * (b - 1) + 1)
                    del h1s[4 * (b - 1)], h1s[4 * (b - 1) + 1]
                    del h1s[4 * (b - 1) + 2], h1s[4 * (b - 1) + 3]
                if b < NB:
                    emit_relu(2 * b)
                    emit_relu(2 * b + 1)

    nc.compile()
    return nc


_NC_CACHE = None


def _get_nc():
    global _NC_CACHE
    if _NC_CACHE is None:
        _NC_CACHE = _build_bass()
    return _NC_CACHE


def _weight_tensors(W1, b1, W2, b2, W3=None):
    w1 = np.ascontiguousarray(W1, dtype=np.float16)
    w2dm = np.concatenate([W2, W2], axis=1).astype(np.float16)
    # w3b[:, 4j+r]: r in {0,1} -> rows 0:64 = W3[:, r]; r in {2,3} ->
    # rows 64:128 = W3[:, r-2]; zero elsewhere.  Identical for every j,
    # so any aligned 4-col slice carries the pair block.
    w3bm = np.zeros((2 * H2, 128), dtype=np.float16)
    for r in range(2):
        w3bm[0:H2, r::4] = W3[:, r:r + 1].astype(np.float16)
        w3bm[H2:2 * H2, r + 2::4] = W3[:, r:r + 1].astype(np.float16)
    b1c = np.ascontiguousarray(b1, dtype=np.float32).reshape(H1, 1)
    b2sc = np.concatenate([b2, b2]).astype(np.float32).reshape(2 * H2, 1)
    return {"w1": w1, "w2d": np.ascontiguousarray(w2dm),
            "w3b": np.ascontiguousarray(w3bm), "b1": b1c, "b2s": b2sc}


def _core_inmap(data, table16, core, weights):
    npad = SEGS * SEGPAIRS * 2 * CHUNK
    dshard = data[core * NPC_RAW:(core + 1) * NPC_RAW]
    dpad = np.zeros((npad, D), dtype=dshard.dtype)
    dpad[:NPC_RAW] = dshard
    # fused fp16 gather + tile-blocked transpose: [SEGS, D, SEGPAIRS*1024]
    xt = np.ascontiguousarray(
        table16[dpad.reshape(SEGS, SEGPAIRS * 2 * CHUNK, D)
                .transpose(0, 2, 1)]
    )
    return {"x_t": xt, **weights}


def _decode_core(arr, arr0, W3f):
    """h2 slabs + packed device outs -> [NPC, 2] f32 outs (no b3)."""
    # slab part: pairs 0..HOSTPAIRS-1
    h2 = (arr.reshape(SLABS, 2, H2, SLABPAIRS, CHUNK)
          .transpose(0, 3, 1, 2, 4)
          .reshape(SLABS * SLABPAIRS, 2, H2, CHUNK)[:HOSTPAIRS])
    outh = np.einsum("phfs,fc->phsc", h2.astype(np.float32), W3f,
                     optimize=True).reshape(HOSTPAIRS * 2 * CHUNK, NCLS)
    # device part: pairs HOSTPAIRS..PAIRS-1
    outd = np.empty((DEVPAIRS * 2 * CHUNK, NCLS), dtype=np.float32)
    for k in range(DEVPAIRS):
        g, m = divmod(k, 3)
        blk = arr0[g, 32 * m:32 * m + 4, :].astype(np.float32)
        s0 = k * 2 * CHUNK
        outd[s0:s0 + CHUNK, 0] = blk[0]
        outd[s0:s0 + CHUNK, 1] = blk[1]
        outd[s0 + CHUNK:s0 + 2 * CHUNK, 0] = blk[2]
        outd[s0 + CHUNK:s0 + 2 * CHUNK, 1] = blk[3]
    return np.concatenate([outh, outd], axis=0)


def _F64(x, y):
    return (
        3.0 * (1.0 - x) ** 2 * np.exp(-(x**2) - (y + 1.0) ** 2)
        - 10.0 * (x / 5.0 - x**3 - y**5) * np.exp(-(x**2) - y**2)
        - 1.0 / (3.0 ** np.exp(-((x + 1.0) ** 2) - y**2))
    )


def kernel(data, embed, W1, b1, W2, b2, W3, b3):
    data = np.asarray(data)
    table16 = np.asarray(embed, dtype=np.float32).reshape(-1).astype(
        np.float16)
    W3f = np.asarray(W3, dtype=np.float32)
    b3c = np.asarray(b3, dtype=np.float32).reshape(NCLS)

    nc = _get_nc()
    weights = _weight_tensors(W1, b1, W2, b2, W3)
    in_maps = [_core_inmap(data, table16, c, weights) for c in range(NCORES)]

    res = run_bass_kernel_spmd(nc, in_maps, core_ids=list(range(NCORES)))

    outs = []
    for c in range(NCORES):
        outs.append(
            _decode_core(res.results[c]["out_d"], res.results[c]["out0_d"],
                         W3f)[:NPC_RAW])
    out_all = np.concatenate(outs, axis=0) + b3c  # [N, 2] fp32

    x64 = out_all[:, 0].astype(np.float64)
    y64 = out_all[:, 1].astype(np.float64)
    pred = _F64(x64, y64)
    idx = int(np.argmax(pred))
    return out_all[idx].astype(np.float32)


# revision 37
# speedup vs baseline: 1.0116x; 1.0116x over previous
"""Trainium2 Bass kernel for nn_NeuralNet_19516331393457 (dense_mlp).

Pipeline: x = embed[data] (48-entry table); h1 = relu(x@W1+b1);
h2 = tanh(h1@W2+b2); out = h2@W3+b3; return out[argmax(F(out0, out1))].

Strategy (data-parallel over N=500000 on 8 cores), fp16 on device:
  - Host: tiny-table gather embed[data] in fp16 fused with a tile-blocked
    transpose; 16 up-front segment DMAs with 8 KiB DRAM lines stream the
    whole shard into SBUF (~124 KiB/partition) at full HBM bandwidth.
  - Device (per core, 63488 padded samples = 62 pairs of 2x512 chunks):
    MM1+relu and pair-packed MM2+tanh for all pairs; the tiny out=h2@W3
    is HYBRID: pairs 0..45 ship h2 (fp16 slabs, host does h2@W3 -- DRAM
    write bandwidth ~254 GB/s absorbs 6 MB while the PE works), pairs
    46..61 run MM3 on-device (their h2 is born too late to drain, so
    they go out as 0.4 MB of packed fp32->fp16 outs instead of 2 MB).
      * software-pipelined, 1-2 iteration skew; in-order PE queue sees
        only aged dependencies; stationaries grouped (w3,w2A,w2B,w1)
      * MM2 pair-packed: chunk A -> PSUM rows 0:64, chunk B -> rows
        64:128 (dup W2 at PE col groups 0/64); one tanh (+b2 stacked)
        evicts both chunks
      * device MM3 pair-packed twice: stacked h2 x block-diag W3 slab ->
        [4,512] at PSUM partition base {0,32,64}; one bank collects 3
        pairs; 6 copies total
      * relu/copies statically balanced between DVE and ACT
  - Host: decode both out paths, out = h2 @ W3 in fp32 for the slab part,
    F in float64, global argmax, return out[argmax] + b3.
"""

import numpy as np

import concourse.mybir as mybir
import concourse.tile as tile
from concourse import bacc
from concourse.bass_utils import run_bass_kernel_spmd

N = 500000
D = 128
H1 = 128
H2 = 64
NCLS = 2
NCORES = 8
CHUNK = 512
NPC_RAW = N // NCORES              # 62500 samples per core
PAIRS = 62                         # pairs of 2 chunks per core
CHUNKS = 2 * PAIRS                 # 124 chunks
NPC = CHUNKS * CHUNK               # 63488 padded samples per core
SEGPAIRS = 4                       # pairs per input DMA segment
SEGS = -(-PAIRS // SEGPAIRS)       # 16 segments (8 KiB DRAM lines each)
HOSTPAIRS = 46                     # pairs whose h2 ships to the host
DEVPAIRS = PAIRS - HOSTPAIRS       # 16 pairs with on-device MM3
SLABPAIRS = 8                      # pairs per output slab
SLABS = -(-HOSTPAIRS // SLABPAIRS)  # 6 slabs (last holds 6 pairs)
GROUPS = -(-DEVPAIRS // 3)         # 6 po groups (3 pairs per PSUM bank)

_F16 = mybir.dt.float16
_F32 = mybir.dt.float32

# measured per-op eviction costs (ns) for static DVE/ACT load balancing
_DVE_RELU, _ACT_RELU = 669.0, 638.0
_DVE_COPY, _ACT_COPY = 700.0, 650.0
_ACT_TANH = 638.0


def _build_bass():
    nc = bacc.Bacc(
        "TRN2",
        target_bir_lowering=False,
        debug=False,
        enable_asserts=False,
        num_devices=NCORES,
    )
    x_t = nc.dram_tensor("x_t", [SEGS, D, SEGPAIRS * 2 * CHUNK], _F16,
                         kind="ExternalInput")
    w1 = nc.dram_tensor("w1", [D, H1], _F16, kind="ExternalInput")
    w2d = nc.dram_tensor("w2d", [H1, 2 * H2], _F16, kind="ExternalInput")
    w3b = nc.dram_tensor("w3b", [2 * H2, 128], _F16, kind="ExternalInput")
    b1 = nc.dram_tensor("b1", [H1, 1], _F32, kind="ExternalInput")
    b2s = nc.dram_tensor("b2s", [2 * H2, 1], _F32, kind="ExternalInput")
    out_d = nc.dram_tensor("out_d", [SLABS, 128, SLABPAIRS * CHUNK], _F16,
                           kind="ExternalOutput")
    out0_d = nc.dram_tensor("out0_d", [GROUPS, 128, CHUNK], _F16,
                            kind="ExternalOutput")

    load = {"dve": 0.0, "act": 0.0}

    with tile.TileContext(nc) as tc:
        with (
            tc.tile_pool(name="w", bufs=1) as wpool,
            tc.tile_pool(name="x", bufs=SEGS) as xpool,
            tc.tile_pool(name="h1", bufs=8) as h1pool,
            tc.tile_pool(name="h2", bufs=4) as h2pool,
            tc.tile_pool(name="st", bufs=SLABS) as stpool,
            tc.tile_pool(name="ob", bufs=GROUPS) as obpool,
            tc.tile_pool(name="p1", bufs=4, space="PSUM") as p1pool,
            tc.tile_pool(name="p2", bufs=2, space="PSUM") as p2pool,
            tc.tile_pool(name="po", bufs=2, space="PSUM") as popool,
        ):
            w1sb = wpool.tile([D, H1], _F16)
            nc.sync.dma_start(w1sb[:], w1[:, :])
            w2sb = wpool.tile([H1, 2 * H2], _F16)
            nc.sync.dma_start(w2sb[:], w2d[:, :])
            w3sb = wpool.tile([2 * H2, 128], _F16)
            nc.sync.dma_start(w3sb[:], w3b[:, :])
            b1sb = wpool.tile([H1, 1], _F32)
            nc.sync.dma_start(b1sb[:], b1[:, :])
            b2sb = wpool.tile([2 * H2, 1], _F32)
            nc.sync.dma_start(b2sb[:], b2s[:, :])

            # prefetch the ACT table set (relu/tanh share one) under the
            # first input DMAs
            warm = wpool.tile([H1, 1], _F32)
            nc.scalar.activation(warm[:], b1sb[:],
                                 mybir.ActivationFunctionType.Relu)

            xsegs = []
            for s in range(SEGS):
                xt = xpool.tile([D, SEGPAIRS * 2 * CHUNK], _F16,
                                name=f"xseg{s}", tag="xt")
                if s == 0:
                    # per-pair slices so the first MM1 starts as soon as
                    # pair 0 lands, not after the whole 1 MiB segment
                    for q in range(SEGPAIRS):
                        sl = slice(q * 2 * CHUNK, (q + 1) * 2 * CHUNK)
                        nc.sync.dma_start(xt[:, sl], x_t[s, :, sl])
                else:
                    nc.sync.dma_start(xt[:], x_t[s, :, :])
                xsegs.append(xt)

            slabs = [
                stpool.tile([128, SLABPAIRS * CHUNK], _F16,
                            name=f"slab{o}", tag="st")
                for o in range(SLABS)
            ]

            p1s = {}     # chunk -> [128, 512] f32 psum
            h1s = {}     # chunk -> [128, 512] f16
            p2s = {}     # pair -> [128, 512] f32 psum (A rows 0:64, B 64:128)
            h2s = {}     # device pair -> [128, 512] f16
            pos = {}     # po group -> [128, 512] f32 psum

            def emit_mm1(p):
                seg, off = divmod(p, SEGPAIRS)
                xt = xsegs[seg]
                for half in range(2):
                    c = 2 * p + half
                    p1 = p1pool.tile([H1, CHUNK], _F32, name=f"p1_{c}",
                                     tag="p1")
                    base = off * 2 * CHUNK + half * CHUNK
                    nc.tensor.matmul(
                        p1[:], w1sb[:], xt[:, base:base + CHUNK],
                        start=True, stop=True)
                    p1s[c] = p1

            def emit_relu(p):
                for half in range(2):
                    c = 2 * p + half
                    h1t = h1pool.tile([H1, CHUNK], _F16, name=f"h1_{c}",
                                      tag="h1")
                    if load["act"] + _ACT_RELU <= load["dve"] + _DVE_RELU:
                        load["act"] += _ACT_RELU
                        nc.scalar.activation(
                            h1t[:], p1s[c][:],
                            mybir.ActivationFunctionType.Relu, bias=b1sb[:])
                    else:
                        load["dve"] += _DVE_RELU
                        nc.vector.tensor_scalar(
                            h1t[:], p1s[c][:], b1sb[:], 0.0,
                            mybir.AluOpType.add, mybir.AluOpType.max)
                    h1s[c] = h1t
                    del p1s[c]

            def emit_mm2(p, col):
                # col 0: chunk A -> rows 0:64; col 1: chunk B -> rows 64:128
                if col == 0:
                    p2 = p2pool.tile([128, CHUNK], _F32, name=f"p2_{p}",
                                     tag="p2")
                    p2s[p] = p2
                nc.tensor.matmul(
                    p2s[p][col * H2:(col + 1) * H2, :],
                    w2sb[:, col * H2:(col + 1) * H2],
                    h1s[2 * p + col][:], start=True, stop=True)

            def emit_tanh(p):
                if p < HOSTPAIRS:
                    o, j = divmod(p, SLABPAIRS)
                    dst = slabs[o][:, j * CHUNK:(j + 1) * CHUNK]
                else:
                    h2t = h2pool.tile([128, CHUNK], _F16, name=f"h2_{p}",
                                      tag="h2")
                    h2s[p] = h2t
                    dst = h2t[:]
                load["act"] += _ACT_TANH
                nc.scalar.activation(
                    dst, p2s[p][:], mybir.ActivationFunctionType.Tanh,
                    bias=b2sb[:])
                del p2s[p]
                if p < HOSTPAIRS and (p == HOSTPAIRS - 1
                                      or j == SLABPAIRS - 1):
                    w = (j + 1) * CHUNK
                    nc.sync.dma_start(out_d[o, :, 0:w], slabs[o][:, 0:w])

            def emit_mm3(p):
                g, m = divmod(p - HOSTPAIRS, 3)
                last = (m == 2) or (p == PAIRS - 1)
                if m == 0:
                    pos[g] = popool.tile([128, CHUNK], _F32,
                                         name=f"po{g}", tag="po")
                nc.tensor.matmul(
                    pos[g][32 * m:32 * m + 4, :],
                    w3sb[:, 32 * m:32 * m + 4], h2s[p][:],
                    start=True, stop=True)
                del h2s[p]
                if last:
                    # full 128-partition staging: the DMA's lines spread
                    # across all 16 queues instead of piling on 0-3
                    ob = obpool.tile([128, CHUNK], _F16, name=f"ob{g}",
                                     tag="ob")
                    if load["act"] + _ACT_COPY <= load["dve"] + _DVE_COPY:
                        load["act"] += _ACT_COPY
                        nc.scalar.copy(ob[:], pos[g][:])
                    else:
                        load["dve"] += _DVE_COPY
                        nc.vector.tensor_copy(ob[:], pos[g][:])
                    nc.sync.dma_start(out0_d[g, :, :], ob[:])
                    del pos[g]

            # per-iteration skew: MM3(b-2) [device pairs], MM2(b-1),
            # MM1(b) on the PE; tanh(b-1) then relu(b) on the eviction
            # engines.  All PE deps are >=1 iteration old.
            NB = PAIRS // 2
            for b in range(NB + 3):
                bm3 = b - 2
                if 0 <= bm3 < NB and 2 * bm3 + 1 >= HOSTPAIRS:
                    for p in (2 * bm3, 2 * bm3 + 1):
                        if p >= HOSTPAIRS:
                            emit_mm3(p)
                if 0 <= b - 1 < NB:
                    for col in range(2):
                        emit_mm2(2 * (b - 1), col)
                        emit_mm2(2 * (b - 1) + 1, col)
                if b < NB:
                    emit_mm1(2 * b)
                    emit_mm1(2 * b + 1)
                if 0 <= b - 1 < NB:
                    emit_tanh(2 * (b - 1))
                    emit_tanh(2 * (b - 1) + 1)
                    del h1s[4 * (b - 1)], h1s[4 * (b - 1) + 1]
                    del h1s[4 * (b - 1) + 2], h1s[4 * (b - 1) + 3]
                if b < NB:
                    emit_relu(2 * b)
                    emit_relu(2 * b + 1)

    nc.compile()
    return nc


_NC_CACHE = None


def _get_nc():
    global _NC_CACHE
    if _NC_CACHE is None:
        _NC_CACHE = _build_bass()
    return _NC_CACHE


def _weight_tensors(W1, b1, W2, b2, W3=None):
    w1 = np.ascontiguousarray(W1, dtype=np.float16)
    w2dm = np.concatenate([W2, W2], axis=1).astype(np.float16)
    # w3b[:, 4j+r]: r in {0,1} -> rows 0:64 = W3[:, r]; r in {2,3} ->
    # rows 64:128 = W3[:, r-2]; zero elsewhere.  Identical for every j,
    # so any aligned 4-col slice carries the pair block.
    w3bm = np.zeros((2 * H2, 128), dtype=np.float16)
    for r in range(2):
        w3bm[0:H2, r::4] = W3[:, r:r + 1].astype(np.float16)
        w3bm[H2:2 * H2, r + 2::4] = W3[:, r:r + 1].astype(np.float16)
    b1c = np.ascontiguousarray(b1, dtype=np.float32).reshape(H1, 1)
    b2sc = np.concatenate([b2, b2]).astype(np.float32).reshape(2 * H2, 1)
    return {"w1": w1, "w2d": np.ascontiguousarray(w2dm),
            "w3b": np.ascontiguousarray(w3bm), "b1": b1c, "b2s": b2sc}


def _core_inmap(data, table16, core, weights):
    npad = SEGS * SEGPAIRS * 2 * CHUNK
    dshard = data[core * NPC_RAW:(core + 1) * NPC_RAW]
    dpad = np.zeros((npad, D), dtype=dshard.dtype)
    dpad[:NPC_RAW] = dshard
    # fused fp16 gather + tile-blocked transpose: [SEGS, D, SEGPAIRS*1024]
    xt = np.ascontiguousarray(
        table16[dpad.reshape(SEGS, SEGPAIRS * 2 * CHUNK, D)
                .transpose(0, 2, 1)]
    )
    return {"x_t": xt, **weights}


def _decode_core(arr, arr0, W3f):
    """h2 slabs + packed device outs -> [NPC, 2] f32 outs (no b3)."""
    # slab part: pairs 0..HOSTPAIRS-1
    h2 = (arr.reshape(SLABS, 2, H2, SLABPAIRS, CHUNK)
          .transpose(0, 3, 1, 2, 4)
          .reshape(SLABS * SLABPAIRS, 2, H2, CHUNK)[:HOSTPAIRS])
    outh = np.einsum("phfs,fc->phsc", h2.astype(np.float32), W3f,
                     optimize=True).reshape(HOSTPAIRS * 2 * CHUNK, NCLS)
    # device part: pairs HOSTPAIRS..PAIRS-1
    outd = np.empty((DEVPAIRS * 2 * CHUNK, NCLS), dtype=np.float32)
    for k in range(DEVPAIRS):
        g, m = divmod(k, 3)
        blk = arr0[g, 32 * m:32 * m + 4, :].astype(np.float32)
        s0 = k * 2 * CHUNK
        outd[s0:s0 + CHUNK, 0] = blk[0]
        outd[s0:s0 + CHUNK, 1] = blk[1]
        outd[s0 + CHUNK:s0 + 2 * CHUNK, 0] = blk[2]
        outd[s0 + CHUNK:s0 + 2 * CHUNK, 1] = blk[3]
    return np.concatenate([outh, outd], axis=0)


def _F64(x, y):
    return (
        3.0 * (1.0 - x) ** 2 * np.exp(-(x**2) - (y + 1.0) ** 2)
        - 10.0 * (x / 5.0 - x**3 - y**5) * np.exp(-(x**2) - y**2)
        - 1.0 / (3.0 ** np.exp(-((x + 1.0) ** 2) - y**2))
    )


def kernel(data, embed, W1, b1, W2, b2, W3, b3):
    data = np.asarray(data)
    table16 = np.asarray(embed, dtype=np.float32).reshape(-1).astype(
        np.float16)
    W3f = np.asarray(W3, dtype=np.float32)
    b3c = np.asarray(b3, dtype=np.float32).reshape(NCLS)

    nc = _get_nc()
    weights = _weight_tensors(W1, b1, W2, b2, W3)
    in_maps = [_core_inmap(data, table16, c, weights) for c in range(NCORES)]

    res = run_bass_kernel_spmd(nc, in_maps, core_ids=list(range(NCORES)))

    outs = []
    for c in range(NCORES):
        outs.append(
            _decode_core(res.results[c]["out_d"], res.results[c]["out0_d"],
                         W3f)[:NPC_RAW])
    out_all = np.concatenate(outs, axis=0) + b3c  # [N, 2] fp32

    x64 = out_all[:, 0].astype(np.float64)
    y64 = out_all[:, 1].astype(np.float64)
    pred = _F64(x64, y64)
    idx = int(np.argmax(pred))
    return out_all[idx].astype(np.float32)
